# revision 13
# baseline (speedup 1.0000x reference)
"""Trainium2 Bass kernel for AliasFreeSampling.

Reference op per (b, c) plane X (512x512):
  reflect-pad 32 -> 65-tap separable lowpass -> 2x2 average pool -> Y (256x256)

The whole per-plane operator is linear and separable, so it folds into a
single 512x256 matrix D (pad + conv + pool combined):  Y = D^T @ X @ D.

On the PE array (out = lhsT.T @ rhs, contraction over partitions):
  phase 1: U^T = X^T @ D    via lhsT = X-chunk   [K=i,128][M=w,128],
                                 rhs = D-chunk   [K=i,128][N=j,win]
           -> U^T [w, j] comes out directly, no transposes anywhere.
  phase 2: Y   = U @ D      via lhsT = U^T-chunk [K=w,128][M=j,128],
                                 rhs = D-chunk   [K=w,128][N=c,win]

D is banded (65-tap filter + 2x pool keeps it local): rows [128k, 128k+128)
only touch ~80-96 of the 256 output columns.  Both phases therefore stream
only each chunk's column window instead of all 256 columns (2112 vs 6144
PE cycles per plane).  PSUM accumulation with differing per-chunk windows
works because start=True arms the whole 2KiB PSUM bank for zero-on-first-
write; consecutive windows overlap, which both covers every column and
gives the Tile scheduler a WAW dep-chain keeping the start=True matmul
first.

x is loaded in natural row order (partition p = row 128*kc + p), so the
contraction chunks are contiguous 128-row blocks; DMA lines are 1 KiB
(f16) which still runs at full DMA throughput (>=512B per descriptor).

Sharding: pure data parallel - 256 (b,c) planes split as 32 planes on each
of the 8 NeuronCores; D is replicated; no cross-core communication.
"""

import numpy as np

import concourse.bacc as bacc
import concourse.bass as bass
import concourse.mybir as mybir
import concourse.tile as tile
from concourse.bass_utils import run_bass_kernel_spmd

N_CORES = 8
N_PLANES = 32        # planes per core
GROUP = 2            # planes per output-DMA batch
H = W = 512
HO = WO = 256
PAD = 32
TAPS = 65

# dtype modes: x input, D filter matrix, U^T intermediate, y output
X_MODE = "f8e3"      # "f16" | "f8e3" (fp8 e3m4: halves input DMA bytes;
                     # PE allows mixed fp8 stationary x f16 moving)
D_MODE = "f16"
UT_MODE = "f16"
Y_MODE = "f16"

_MYBIR_DT = {
    "f16": mybir.dt.float16,
    "bf16": mybir.dt.bfloat16,
    "f8e3": mybir.dt.float8e3,
    "f32": mybir.dt.float32,
}


def _np_dt(mode):
    if mode == "f16":
        return np.float16
    if mode == "f32":
        return np.float32
    import ml_dtypes
    return {"bf16": ml_dtypes.bfloat16, "f8e3": ml_dtypes.float8_e3m4}[mode]


def _make_D(k: np.ndarray) -> np.ndarray:
    """Fold reflect-pad(32) + 65-tap conv + 2x avg-pool into one 512x256 map."""
    assert k.shape == (TAPS,)
    D = np.zeros((H, HO), dtype=np.float64)
    t = np.arange(TAPS)
    for j in range(HO):
        for r in (2 * j, 2 * j + 1):
            q = r + t - PAD
            i = np.where(q < 0, -q, np.where(q >= H, 2 * H - 2 - q, q))
            np.add.at(D[:, j], i, 0.5 * k.astype(np.float64))
    return D.astype(np.float32)


def _chunk_windows():
    """Per 128-row chunk of D, the even-aligned column support window."""
    Dp = _make_D(np.ones(TAPS, dtype=np.float32))
    wins = []
    for c in range(4):
        nz = np.nonzero(np.any(Dp[c * 128:(c + 1) * 128] != 0.0, axis=0))[0]
        j0 = int(nz.min()) & ~1
        j1 = min(HO, (int(nz.max()) + 2) & ~1)
        wins.append((j0, j1))
    cover = 0
    for a, b in zip(wins, wins[1:]):
        assert b[0] < a[1], f"windows must overlap for ordering: {wins}"
    assert wins[0][0] == 0 and wins[-1][1] == HO
    return wins


def _emit(tc, y, x, d, n_planes):
    nc = tc.nc
    f32 = mybir.dt.float32
    WIN = _chunk_windows()

    from contextlib import ExitStack
    with ExitStack() as ctx:
        xpool = ctx.enter_context(tc.tile_pool(name="xin", bufs=3))
        dpool = ctx.enter_context(tc.tile_pool(name="dconst", bufs=1))
        utpool = ctx.enter_context(tc.tile_pool(name="ut", bufs=4))
        ypool = ctx.enter_context(tc.tile_pool(name="yout", bufs=3))
        pspool = ctx.enter_context(tc.tile_pool(name="ps", bufs=1, space="PSUM"))

        # D in natural 128-row chunks: partition p = row 128*kc + p.  Serves
        # as the rhs of both phases (phase 1 contracts rows i, phase 2 rows w).
        d_sb = dpool.tile([128, 4, HO], d.dtype)
        nc.scalar.dma_start(d_sb[:], d.rearrange("(kc p) j -> p kc j", p=128))

        ut_dt = _MYBIR_DT[UT_MODE]

        # PSUM->SBUF casts carry a ~125-290ns fixed access latency per
        # instruction, so batch each plane's phase-1 result into ONE 2-bank
        # PSUM tile + ONE ACT cast, and each plane's phase-2 result into ONE
        # 1-bank tile + ONE DVE cast.  start=True arms the whole 2KiB bank
        # (lazy zero-on-first-matmul-write), so only the first sub-matmul of
        # each bank carries start=True and only the last carries stop=True.
        y_sbs = {}

        def phase1(p):
            # x is pre-permuted on the host to [plane, q, kc, w] so each
            # partition line is one 2KiB contiguous DRAM run (128
            # descriptors per plane instead of 512).
            xn = xpool.tile([128, 4, W], x.dtype, tag="x", bufs=12)
            xv = x[p]
            if p < 2:
                # head: per-chunk loads so the first matmul starts after
                # 64 KiB instead of the whole plane
                for kc in range(4):
                    nc.sync.dma_start(xn[:, kc], xv[:, kc])
            else:
                nc.sync.dma_start(xn[:], xv)

            ut_ps = pspool.tile([128, 4, HO], f32, tag="utps", bufs=3)
            for wc in range(4):
                for kc in range(4):
                    j0, j1 = WIN[kc]
                    nc.tensor.matmul(
                        ut_ps[:, wc, j0:j1],
                        xn[:, kc, wc * 128:(wc + 1) * 128],
                        d_sb[:, kc, j0:j1],
                        # banks are (wc 0,1) and (wc 2,3)
                        start=(kc == 0 and wc in (0, 2)),
                        stop=(kc == 3 and wc in (1, 3)),
                    )
            ut = utpool.tile([128, 4, HO], ut_dt, tag="ut")
            nc.scalar.copy(ut[:], ut_ps[:])
            return ut

        def phase2(p, ut):
            g, pl = divmod(p, GROUP)
            if pl == 0:
                y_sbs[g] = ypool.tile([128, GROUP, 2, WO], y.dtype, tag="y",
                                      name=f"y_sb{g}")
            y_sb = y_sbs[g]
            utv = ut[:].rearrange("q wc (j2 rr) -> q wc j2 rr", rr=2)
            y_ps = pspool.tile([128, 2, WO], f32, tag="yps", bufs=2)
            for rr in range(2):
                for wc in range(4):
                    j0, j1 = WIN[wc]
                    nc.tensor.matmul(
                        y_ps[:, rr, j0:j1],
                        utv[:, wc, :, rr],
                        d_sb[:, wc, j0:j1],
                        start=(wc == 0 and rr == 0),
                        stop=(wc == 3 and rr == 1),
                    )
            nc.vector.tensor_copy(y_sb[:, pl, :, :], y_ps[:])
            if pl == GROUP - 1:
                # y DRAM carries the SBUF layout (un-permuted on the host):
                # one 2KiB contiguous run per partition per store.
                nc.sync.dma_start(y[g], y_sb[:])
                del y_sbs[g]

        # software pipeline: phase2(p-1) is emitted between phase1(p) and
        # phase1(p+1) so the PE never stalls waiting for plane p's ut cast.
        prev = None
        for p in range(n_planes):
            ut = phase1(p)
            if prev is not None:
                phase2(prev[0], prev[1])
            prev = (p, ut)
        phase2(prev[0], prev[1])


def build_nc(n_planes=N_PLANES):
    nc = bacc.Bacc("TRN2", target_bir_lowering=False, debug=False)
    x = nc.dram_tensor("x", [n_planes, 128, 4, W], _MYBIR_DT[X_MODE],
                       kind="ExternalInput").ap()
    d = nc.dram_tensor("d", [H, HO], _MYBIR_DT[D_MODE],
                       kind="ExternalInput").ap()
    y = nc.dram_tensor("y", [n_planes // GROUP, 128, GROUP, 2, WO],
                       _MYBIR_DT[Y_MODE], kind="ExternalOutput").ap()
    with tile.TileContext(nc) as tc:
        _emit(tc, y, x, d, n_planes)
    nc.compile()
    return nc


_NC_CACHE = {}


def _get_nc(n_planes=N_PLANES):
    key = (n_planes, X_MODE, D_MODE, UT_MODE, Y_MODE)
    if key not in _NC_CACHE:
        _NC_CACHE[key] = build_nc(n_planes)
    return _NC_CACHE[key]


def kernel(x, kernel, **run_kwargs):
    x = np.asarray(x, dtype=np.float32)
    k = np.asarray(kernel, dtype=np.float32)
    B, C = x.shape[0], x.shape[1]
    assert x.shape == (B, C, H, W) and B * C == N_CORES * N_PLANES

    nc = _get_nc()
    d_in = _make_D(k).astype(_np_dt(D_MODE))
    # pre-permute x into the kernel's DMA-native layout:
    # [plane, q, kc, w] with row (128*kc + q) -> 2KiB contiguous per (plane, q)
    xs = x.reshape(N_CORES * N_PLANES, 4, 128, W).transpose(0, 2, 1, 3)
    xs = np.ascontiguousarray(xs, dtype=np.float32).astype(_np_dt(X_MODE))
    in_maps = [
        {"x": xs[c * N_PLANES:(c + 1) * N_PLANES], "d": d_in}
        for c in range(N_CORES)
    ]
    res = run_bass_kernel_spmd(nc, in_maps, core_ids=list(range(N_CORES)), **run_kwargs)
    # y arrives as [n_planes//GROUP, q, pl, rr, c]; un-permute on the host:
    # plane = GROUP*g + pl, row = 2*q + rr
    y = np.stack([np.asarray(r["y"], dtype=np.float32) for r in res.results])
    y = y.transpose(0, 1, 3, 2, 4, 5).reshape(N_CORES * N_PLANES, 128, 2, WO)
    out = np.ascontiguousarray(y).reshape(B, C, HO, WO)
    if run_kwargs:
        return out, res
    return out


# revision 14
# speedup vs baseline: 1.0989x; 1.0989x over previous
"""Trainium2 Bass kernel for AliasFreeSampling.

Reference op per (b, c) plane X (512x512):
  reflect-pad 32 -> 65-tap separable lowpass -> 2x2 average pool -> Y (256x256)

The whole per-plane operator is linear and separable, so it folds into a
single 512x256 matrix D (pad + conv + pool combined):  Y = D^T @ X @ D.

On the PE array (out = lhsT.T @ rhs, contraction over partitions):
  phase 1: U^T = X^T @ D    via lhsT = X-chunk   [K=i,128][M=w,128],
                                 rhs = D-chunk   [K=i,128][N=j,win]
           -> U^T [w, j] comes out directly, no transposes anywhere.
  phase 2: Y   = U @ D      via lhsT = U^T-chunk [K=w,128][M=j,128],
                                 rhs = D-chunk   [K=w,128][N=c,win]

D is banded (65-tap filter + 2x pool keeps it local): rows [128k, 128k+128)
only touch ~80-96 of the 256 output columns.  Both phases therefore stream
only each chunk's column window instead of all 256 columns (2112 vs 6144
PE cycles per plane).  PSUM accumulation with differing per-chunk windows
works because start=True arms the whole 2KiB PSUM bank for zero-on-first-
write; consecutive windows overlap, which both covers every column and
gives the Tile scheduler a WAW dep-chain keeping the start=True matmul
first.

x is loaded in natural row order (partition p = row 128*kc + p), so the
contraction chunks are contiguous 128-row blocks; DMA lines are 1 KiB
(f16) which still runs at full DMA throughput (>=512B per descriptor).

Sharding: pure data parallel - 256 (b,c) planes split as 32 planes on each
of the 8 NeuronCores; D is replicated; no cross-core communication.
"""

import numpy as np

import concourse.bacc as bacc
import concourse.bass as bass
import concourse.mybir as mybir
import concourse.tile as tile
from concourse.bass_utils import run_bass_kernel_spmd

N_CORES = 8
N_PLANES = 32        # planes per core
GROUP = 4            # planes per output-DMA batch
XPAIR = 2            # planes per input-DMA batch
H = W = 512
HO = WO = 256
PAD = 32
TAPS = 65

# dtype modes: x input, D filter matrix, U^T intermediate, y output
X_MODE = "f8e3"      # "f16" | "f8e3" (fp8 e3m4: halves input DMA bytes;
                     # PE allows mixed fp8 stationary x f16 moving)
D_MODE = "f16"
UT_MODE = "f16"
Y_MODE = "f16"

_MYBIR_DT = {
    "f16": mybir.dt.float16,
    "bf16": mybir.dt.bfloat16,
    "f8e3": mybir.dt.float8e3,
    "f32": mybir.dt.float32,
}


def _np_dt(mode):
    if mode == "f16":
        return np.float16
    if mode == "f32":
        return np.float32
    import ml_dtypes
    return {"bf16": ml_dtypes.bfloat16, "f8e3": ml_dtypes.float8_e3m4}[mode]


def _make_D(k: np.ndarray) -> np.ndarray:
    """Fold reflect-pad(32) + 65-tap conv + 2x avg-pool into one 512x256 map."""
    assert k.shape == (TAPS,)
    D = np.zeros((H, HO), dtype=np.float64)
    t = np.arange(TAPS)
    for j in range(HO):
        for r in (2 * j, 2 * j + 1):
            q = r + t - PAD
            i = np.where(q < 0, -q, np.where(q >= H, 2 * H - 2 - q, q))
            np.add.at(D[:, j], i, 0.5 * k.astype(np.float64))
    return D.astype(np.float32)


def _chunk_windows():
    """Per 128-row chunk of D, the even-aligned column support window."""
    Dp = _make_D(np.ones(TAPS, dtype=np.float32))
    wins = []
    for c in range(4):
        nz = np.nonzero(np.any(Dp[c * 128:(c + 1) * 128] != 0.0, axis=0))[0]
        j0 = int(nz.min()) & ~1
        j1 = min(HO, (int(nz.max()) + 2) & ~1)
        wins.append((j0, j1))
    cover = 0
    for a, b in zip(wins, wins[1:]):
        assert b[0] < a[1], f"windows must overlap for ordering: {wins}"
    assert wins[0][0] == 0 and wins[-1][1] == HO
    return wins


def _emit(tc, y, x, d, n_planes):
    nc = tc.nc
    f32 = mybir.dt.float32
    WIN = _chunk_windows()

    from contextlib import ExitStack
    with ExitStack() as ctx:
        xpool = ctx.enter_context(tc.tile_pool(name="xin", bufs=3))
        dpool = ctx.enter_context(tc.tile_pool(name="dconst", bufs=1))
        utpool = ctx.enter_context(tc.tile_pool(name="ut", bufs=4))
        ypool = ctx.enter_context(tc.tile_pool(name="yout", bufs=3))
        pspool = ctx.enter_context(tc.tile_pool(name="ps", bufs=1, space="PSUM"))

        # D in natural 128-row chunks: partition p = row 128*kc + p.  Serves
        # as the rhs of both phases (phase 1 contracts rows i, phase 2 rows w).
        d_sb = dpool.tile([128, 4, HO], d.dtype)
        nc.scalar.dma_start(d_sb[:], d.rearrange("(kc p) j -> p kc j", p=128))

        ut_dt = _MYBIR_DT[UT_MODE]

        # PSUM->SBUF casts carry a ~125-290ns fixed access latency per
        # instruction, so batch each plane's phase-1 result into ONE 2-bank
        # PSUM tile + ONE ACT cast, and each plane's phase-2 result into ONE
        # 1-bank tile + ONE DVE cast.  start=True arms the whole 2KiB bank
        # (lazy zero-on-first-matmul-write), so only the first sub-matmul of
        # each bank carries start=True and only the last carries stop=True.
        y_sbs = {}

        xns = {}

        def phase1(p):
            # x is pre-permuted on the host to [plane, q, kc, w] so each
            # partition line is one 2KiB contiguous DRAM run (128
            # descriptors per plane instead of 512); planes are loaded in
            # pairs to halve the fixed ~740ns dma_start issue cost.
            pp, pl2 = divmod(p, XPAIR)
            if pl2 == 0:
                xns[pp] = xpool.tile([128, XPAIR, 4, W], x.dtype, tag="x",
                                     bufs=6, name=f"xn{pp}")
                xv = x[pp * XPAIR:(pp + 1) * XPAIR].rearrange(
                    "pl q kc w -> q pl kc w")
                if pp == 0:
                    # head: smaller loads so the first matmul starts sooner
                    for h in range(XPAIR):
                        for kc in range(4):
                            nc.sync.dma_start(xns[pp][:, h, kc], xv[:, h, kc])
                else:
                    nc.sync.dma_start(xns[pp][:], xv)
            xn = xns[pp]

            ut_ps = pspool.tile([128, 4, HO], f32, tag="utps", bufs=3)
            for wc in range(4):
                for kc in range(4):
                    j0, j1 = WIN[kc]
                    nc.tensor.matmul(
                        ut_ps[:, wc, j0:j1],
                        xn[:, pl2, kc, wc * 128:(wc + 1) * 128],
                        d_sb[:, kc, j0:j1],
                        # banks are (wc 0,1) and (wc 2,3)
                        start=(kc == 0 and wc in (0, 2)),
                        stop=(kc == 3 and wc in (1, 3)),
                    )
            ut = utpool.tile([128, 4, HO], ut_dt, tag="ut")
            # split the PSUM->SBUF cast ~3:1 between ACT and DVE
            nc.scalar.copy(ut[:, 0:3, :], ut_ps[:, 0:3, :])
            nc.vector.tensor_copy(ut[:, 3, :], ut_ps[:, 3, :])
            return ut

        def phase2(p, ut):
            g, pl = divmod(p, GROUP)
            if pl == 0:
                y_sbs[g] = ypool.tile([128, GROUP, 2, WO], y.dtype, tag="y",
                                      name=f"y_sb{g}")
            y_sb = y_sbs[g]
            utv = ut[:].rearrange("q wc (j2 rr) -> q wc j2 rr", rr=2)
            y_ps = pspool.tile([128, 2, WO], f32, tag="yps", bufs=2)
            for rr in range(2):
                for wc in range(4):
                    j0, j1 = WIN[wc]
                    nc.tensor.matmul(
                        y_ps[:, rr, j0:j1],
                        utv[:, wc, :, rr],
                        d_sb[:, wc, j0:j1],
                        start=(wc == 0 and rr == 0),
                        stop=(wc == 3 and rr == 1),
                    )
            nc.vector.tensor_copy(y_sb[:, pl, :, :], y_ps[:])
            if pl == GROUP - 1:
                # y DRAM carries the SBUF layout (un-permuted on the host):
                # one 2KiB contiguous run per partition per store.
                nc.sync.dma_start(y[g], y_sb[:])
                del y_sbs[g]

        # software pipeline: phase2(p-1) is emitted between phase1(p) and
        # phase1(p+1) so the PE never stalls waiting for plane p's ut cast.
        prev = None
        for p in range(n_planes):
            ut = phase1(p)
            if prev is not None:
                phase2(prev[0], prev[1])
            prev = (p, ut)
        phase2(prev[0], prev[1])


def build_nc(n_planes=N_PLANES):
    nc = bacc.Bacc("TRN2", target_bir_lowering=False, debug=False)
    x = nc.dram_tensor("x", [n_planes, 128, 4, W], _MYBIR_DT[X_MODE],
                       kind="ExternalInput").ap()
    d = nc.dram_tensor("d", [H, HO], _MYBIR_DT[D_MODE],
                       kind="ExternalInput").ap()
    y = nc.dram_tensor("y", [n_planes // GROUP, 128, GROUP, 2, WO],
                       _MYBIR_DT[Y_MODE], kind="ExternalOutput").ap()
    with tile.TileContext(nc) as tc:
        _emit(tc, y, x, d, n_planes)
    nc.compile()
    return nc


_NC_CACHE = {}


def _get_nc(n_planes=N_PLANES):
    key = (n_planes, X_MODE, D_MODE, UT_MODE, Y_MODE)
    if key not in _NC_CACHE:
        _NC_CACHE[key] = build_nc(n_planes)
    return _NC_CACHE[key]


def kernel(x, kernel, **run_kwargs):
    x = np.asarray(x, dtype=np.float32)
    k = np.asarray(kernel, dtype=np.float32)
    B, C = x.shape[0], x.shape[1]
    assert x.shape == (B, C, H, W) and B * C == N_CORES * N_PLANES

    nc = _get_nc()
    d_in = _make_D(k).astype(_np_dt(D_MODE))
    # pre-permute x into the kernel's DMA-native layout:
    # [plane, q, kc, w] with row (128*kc + q) -> 2KiB contiguous per (plane, q)
    xs = x.reshape(N_CORES * N_PLANES, 4, 128, W).transpose(0, 2, 1, 3)
    xs = np.ascontiguousarray(xs, dtype=np.float32).astype(_np_dt(X_MODE))
    in_maps = [
        {"x": xs[c * N_PLANES:(c + 1) * N_PLANES], "d": d_in}
        for c in range(N_CORES)
    ]
    res = run_bass_kernel_spmd(nc, in_maps, core_ids=list(range(N_CORES)), **run_kwargs)
    # y arrives as [n_planes//GROUP, q, pl, rr, c]; un-permute on the host:
    # plane = GROUP*g + pl, row = 2*q + rr
    y = np.stack([np.asarray(r["y"], dtype=np.float32) for r in res.results])
    y = y.transpose(0, 1, 3, 2, 4, 5).reshape(N_CORES * N_PLANES, 128, 2, WO)
    out = np.ascontiguousarray(y).reshape(B, C, HO, WO)
    if run_kwargs:
        return out, res
    return out


# revision 15
# speedup vs baseline: 1.1776x; 1.0716x over previous
"""Trainium2 Bass kernel for AliasFreeSampling.

Reference op per (b, c) plane X (512x512):
  reflect-pad 32 -> 65-tap separable lowpass -> 2x2 average pool -> Y (256x256)

The whole per-plane operator is linear and separable, so it folds into a
single 512x256 matrix D (pad + conv + pool combined):  Y = D^T @ X @ D.

On the PE array (out = lhsT.T @ rhs, contraction over partitions):
  phase 1: U^T = X^T @ D    via lhsT = X-chunk   [K=i,128][M=w,128],
                                 rhs = D-chunk   [K=i,128][N=j,win]
           -> U^T [w, j] comes out directly, no transposes anywhere.
  phase 2: Y   = U @ D      via lhsT = U^T-chunk [K=w,128][M=j,128],
                                 rhs = D-chunk   [K=w,128][N=c,win]

D is banded (65-tap filter + 2x pool keeps it local): rows [128k, 128k+128)
only touch ~80-96 of the 256 output columns.  Both phases therefore stream
only each chunk's column window instead of all 256 columns (2112 vs 6144
PE cycles per plane).  PSUM accumulation with differing per-chunk windows
works because start=True arms the whole 2KiB PSUM bank for zero-on-first-
write; consecutive windows overlap, which both covers every column and
gives the Tile scheduler a WAW dep-chain keeping the start=True matmul
first.

x is loaded in natural row order (partition p = row 128*kc + p), so the
contraction chunks are contiguous 128-row blocks; DMA lines are 1 KiB
(f16) which still runs at full DMA throughput (>=512B per descriptor).

Sharding: pure data parallel - 256 (b,c) planes split as 32 planes on each
of the 8 NeuronCores; D is replicated; no cross-core communication.
"""

import numpy as np

import concourse.bacc as bacc
import concourse.bass as bass
import concourse.mybir as mybir
import concourse.tile as tile
from concourse.bass_utils import run_bass_kernel_spmd

N_CORES = 8
N_PLANES = 32        # planes per core
GROUP = 4            # planes per output-DMA batch
XPAIR = 2            # planes per input-DMA batch
H = W = 512
HO = WO = 256
PAD = 32
TAPS = 65

# dtype modes: x input, D filter matrix, U^T intermediate, y output
X_MODE = "f8e3"      # "f16" | "f8e3" (fp8 e3m4: halves input DMA bytes;
                     # PE allows mixed fp8 stationary x f16 moving)
D_MODE = "f16"
UT_MODE = "f16"
Y_MODE = "f16"

_MYBIR_DT = {
    "f16": mybir.dt.float16,
    "bf16": mybir.dt.bfloat16,
    "f8e3": mybir.dt.float8e3,
    "f32": mybir.dt.float32,
}


def _np_dt(mode):
    if mode == "f16":
        return np.float16
    if mode == "f32":
        return np.float32
    import ml_dtypes
    return {"bf16": ml_dtypes.bfloat16, "f8e3": ml_dtypes.float8_e3m4}[mode]


def _make_D(k: np.ndarray) -> np.ndarray:
    """Fold reflect-pad(32) + 65-tap conv + 2x avg-pool into one 512x256 map."""
    assert k.shape == (TAPS,)
    D = np.zeros((H, HO), dtype=np.float64)
    t = np.arange(TAPS)
    for j in range(HO):
        for r in (2 * j, 2 * j + 1):
            q = r + t - PAD
            i = np.where(q < 0, -q, np.where(q >= H, 2 * H - 2 - q, q))
            np.add.at(D[:, j], i, 0.5 * k.astype(np.float64))
    return D.astype(np.float32)


def _chunk_windows():
    """Per 128-row chunk of D, the even-aligned column support window."""
    Dp = _make_D(np.ones(TAPS, dtype=np.float32))
    wins = []
    for c in range(4):
        nz = np.nonzero(np.any(Dp[c * 128:(c + 1) * 128] != 0.0, axis=0))[0]
        j0 = int(nz.min()) & ~1
        j1 = min(HO, (int(nz.max()) + 2) & ~1)
        wins.append((j0, j1))
    cover = 0
    for a, b in zip(wins, wins[1:]):
        assert b[0] < a[1], f"windows must overlap for ordering: {wins}"
    assert wins[0][0] == 0 and wins[-1][1] == HO
    return wins


def _emit(tc, y, x, d, n_planes):
    nc = tc.nc
    f32 = mybir.dt.float32
    WIN = _chunk_windows()

    from contextlib import ExitStack
    with ExitStack() as ctx:
        xpool = ctx.enter_context(tc.tile_pool(name="xin", bufs=3))
        dpool = ctx.enter_context(tc.tile_pool(name="dconst", bufs=1))
        utpool = ctx.enter_context(tc.tile_pool(name="ut", bufs=4))
        ypool = ctx.enter_context(tc.tile_pool(name="yout", bufs=3))
        pspool = ctx.enter_context(tc.tile_pool(name="ps", bufs=1, space="PSUM"))

        # D in natural 128-row chunks: partition p = row 128*kc + p.  Serves
        # as the rhs of both phases (phase 1 contracts rows i, phase 2 rows w).
        d_sb = dpool.tile([128, 4, HO], d.dtype)
        nc.scalar.dma_start(d_sb[:], d.rearrange("(kc p) j -> p kc j", p=128))

        ut_dt = _MYBIR_DT[UT_MODE]

        # PSUM->SBUF casts carry a ~125-290ns fixed access latency per
        # instruction, so batch each plane's phase-1 result into ONE 2-bank
        # PSUM tile + ONE ACT cast, and each plane's phase-2 result into ONE
        # 1-bank tile + ONE DVE cast.  start=True arms the whole 2KiB bank
        # (lazy zero-on-first-matmul-write), so only the first sub-matmul of
        # each bank carries start=True and only the last carries stop=True.
        y_sbs = {}

        xns = {}

        def phase1(p):
            # x is pre-permuted on the host to [plane, q, kc, w] so each
            # partition line is one 2KiB contiguous DRAM run (128
            # descriptors per plane instead of 512); planes are loaded in
            # pairs to halve the fixed ~740ns dma_start issue cost.
            pp, pl2 = divmod(p, XPAIR)
            if pl2 == 0:
                xns[pp] = xpool.tile([128, XPAIR, 4, W], x.dtype, tag="x",
                                     bufs=6, name=f"xn{pp}")
                xv = x[pp * XPAIR:(pp + 1) * XPAIR].rearrange(
                    "pl q kc w -> q pl kc w")
                if pp == 0:
                    # head: smaller loads so the first matmul starts sooner
                    for h in range(XPAIR):
                        for kc in range(4):
                            nc.sync.dma_start(xns[pp][:, h, kc], xv[:, h, kc])
                else:
                    nc.sync.dma_start(xns[pp][:], xv)
            xn = xns[pp]

            ut_ps = pspool.tile([128, 4, HO], f32, tag="utps", bufs=3)
            for wc in range(4):
                for kc in range(4):
                    j0, j1 = WIN[kc]
                    nc.tensor.matmul(
                        ut_ps[:, wc, j0:j1],
                        xn[:, pl2, kc, wc * 128:(wc + 1) * 128],
                        d_sb[:, kc, j0:j1],
                        # banks are (wc 0,1) and (wc 2,3)
                        start=(kc == 0 and wc in (0, 2)),
                        stop=(kc == 3 and wc in (1, 3)),
                    )
            ut = utpool.tile([128, 4, HO], ut_dt, tag="ut")
            # split the PSUM->SBUF cast ~3:1 between ACT and DVE
            nc.scalar.copy(ut[:, 0:3, :], ut_ps[:, 0:3, :])
            nc.vector.tensor_copy(ut[:, 3, :], ut_ps[:, 3, :])
            return ut

        def phase2(p, ut):
            g, pl = divmod(p, GROUP)
            if pl == 0:
                y_sbs[g] = ypool.tile([128, GROUP, 2, WO], y.dtype, tag="y",
                                      name=f"y_sb{g}")
            y_sb = y_sbs[g]
            utv = ut[:].rearrange("q wc (j2 rr) -> q wc j2 rr", rr=2)
            y_ps = pspool.tile([128, 2, WO], f32, tag="yps", bufs=2)
            for rr in range(2):
                for wc in range(4):
                    j0, j1 = WIN[wc]
                    nc.tensor.matmul(
                        y_ps[:, rr, j0:j1],
                        utv[:, wc, :, rr],
                        d_sb[:, wc, j0:j1],
                        start=(wc == 0 and rr == 0),
                        stop=(wc == 3 and rr == 1),
                    )
            nc.vector.tensor_copy(y_sb[:, pl, :, :], y_ps[:])
            if pl == GROUP - 1:
                # y DRAM carries the SBUF layout (un-permuted on the host):
                # one 2KiB contiguous run per partition per store.
                nc.sync.dma_start(y[g], y_sb[:])
                del y_sbs[g]

        # software pipeline, distance 2: phase2(p-2) is emitted between
        # phase1(p) and phase1(p+1), giving plane p's ut cast two full
        # phase-1 blocks (~2.6us) to complete before the PE needs it.
        DIST = 2
        pending = []
        for p in range(n_planes):
            ut = phase1(p)
            pending.append((p, ut))
            if len(pending) > DIST:
                pp_, ut_ = pending.pop(0)
                phase2(pp_, ut_)
        for pp_, ut_ in pending:
            phase2(pp_, ut_)


def build_nc(n_planes=N_PLANES):
    nc = bacc.Bacc("TRN2", target_bir_lowering=False, debug=False)
    x = nc.dram_tensor("x", [n_planes, 128, 4, W], _MYBIR_DT[X_MODE],
                       kind="ExternalInput").ap()
    d = nc.dram_tensor("d", [H, HO], _MYBIR_DT[D_MODE],
                       kind="ExternalInput").ap()
    y = nc.dram_tensor("y", [n_planes // GROUP, 128, GROUP, 2, WO],
                       _MYBIR_DT[Y_MODE], kind="ExternalOutput").ap()
    with tile.TileContext(nc) as tc:
        _emit(tc, y, x, d, n_planes)
    nc.compile()
    return nc


_NC_CACHE = {}


def _get_nc(n_planes=N_PLANES):
    key = (n_planes, X_MODE, D_MODE, UT_MODE, Y_MODE)
    if key not in _NC_CACHE:
        _NC_CACHE[key] = build_nc(n_planes)
    return _NC_CACHE[key]


def kernel(x, kernel, **run_kwargs):
    x = np.asarray(x, dtype=np.float32)
    k = np.asarray(kernel, dtype=np.float32)
    B, C = x.shape[0], x.shape[1]
    assert x.shape == (B, C, H, W) and B * C == N_CORES * N_PLANES

    nc = _get_nc()
    d_in = _make_D(k).astype(_np_dt(D_MODE))
    # pre-permute x into the kernel's DMA-native layout:
    # [plane, q, kc, w] with row (128*kc + q) -> 2KiB contiguous per (plane, q)
    xs = x.reshape(N_CORES * N_PLANES, 4, 128, W).transpose(0, 2, 1, 3)
    xs = np.ascontiguousarray(xs, dtype=np.float32).astype(_np_dt(X_MODE))
    in_maps = [
        {"x": xs[c * N_PLANES:(c + 1) * N_PLANES], "d": d_in}
        for c in range(N_CORES)
    ]
    res = run_bass_kernel_spmd(nc, in_maps, core_ids=list(range(N_CORES)), **run_kwargs)
    # y arrives as [n_planes//GROUP, q, pl, rr, c]; un-permute on the host:
    # plane = GROUP*g + pl, row = 2*q + rr
    y = np.stack([np.asarray(r["y"], dtype=np.float32) for r in res.results])
    y = y.transpose(0, 1, 3, 2, 4, 5).reshape(N_CORES * N_PLANES, 128, 2, WO)
    out = np.ascontiguousarray(y).reshape(B, C, HO, WO)
    if run_kwargs:
        return out, res
    return out


# revision 16
# speedup vs baseline: 1.1961x; 1.0157x over previous
"""Trainium2 Bass kernel for AliasFreeSampling.

Reference op per (b, c) plane X (512x512):
  reflect-pad 32 -> 65-tap separable lowpass -> 2x2 average pool -> Y (256x256)

The whole per-plane operator is linear and separable, so it folds into a
single 512x256 matrix D (pad + conv + pool combined):  Y = D^T @ X @ D.

On the PE array (out = lhsT.T @ rhs, contraction over partitions):
  phase 1: U^T = X^T @ D    via lhsT = X-chunk   [K=i,128][M=w,128],
                                 rhs = D-chunk   [K=i,128][N=j,win]
           -> U^T [w, j] comes out directly, no transposes anywhere.
  phase 2: Y   = U @ D      via lhsT = U^T-chunk [K=w,128][M=j,128],
                                 rhs = D-chunk   [K=w,128][N=c,win]

D is banded (65-tap filter + 2x pool keeps it local): rows [128k, 128k+128)
only touch ~80-96 of the 256 output columns.  Both phases therefore stream
only each chunk's column window instead of all 256 columns (2112 vs 6144
PE cycles per plane).  PSUM accumulation with differing per-chunk windows
works because start=True arms the whole 2KiB PSUM bank for zero-on-first-
write; consecutive windows overlap, which both covers every column and
gives the Tile scheduler a WAW dep-chain keeping the start=True matmul
first.

x is loaded in natural row order (partition p = row 128*kc + p), so the
contraction chunks are contiguous 128-row blocks; DMA lines are 1 KiB
(f16) which still runs at full DMA throughput (>=512B per descriptor).

Sharding: pure data parallel - 256 (b,c) planes split as 32 planes on each
of the 8 NeuronCores; D is replicated; no cross-core communication.
"""

import numpy as np

import concourse.bacc as bacc
import concourse.bass as bass
import concourse.mybir as mybir
import concourse.tile as tile
from concourse.bass_utils import run_bass_kernel_spmd

N_CORES = 8
N_PLANES = 32        # planes per core
GROUP = 4            # planes per output-DMA batch
XPAIR = 2            # planes per input-DMA batch
H = W = 512
HO = WO = 256
PAD = 32
TAPS = 65

# dtype modes: x input, D filter matrix, U^T intermediate, y output
X_MODE = "f8e3"      # "f16" | "f8e3" (fp8 e3m4: halves input DMA bytes;
                     # PE allows mixed fp8 stationary x f16 moving)
D_MODE = "f16"
UT_MODE = "f16"
Y_MODE = "f16"

_MYBIR_DT = {
    "f16": mybir.dt.float16,
    "bf16": mybir.dt.bfloat16,
    "f8e3": mybir.dt.float8e3,
    "f32": mybir.dt.float32,
}


def _np_dt(mode):
    if mode == "f16":
        return np.float16
    if mode == "f32":
        return np.float32
    import ml_dtypes
    return {"bf16": ml_dtypes.bfloat16, "f8e3": ml_dtypes.float8_e3m4}[mode]


def _make_D(k: np.ndarray) -> np.ndarray:
    """Fold reflect-pad(32) + 65-tap conv + 2x avg-pool into one 512x256 map."""
    assert k.shape == (TAPS,)
    D = np.zeros((H, HO), dtype=np.float64)
    t = np.arange(TAPS)
    for j in range(HO):
        for r in (2 * j, 2 * j + 1):
            q = r + t - PAD
            i = np.where(q < 0, -q, np.where(q >= H, 2 * H - 2 - q, q))
            np.add.at(D[:, j], i, 0.5 * k.astype(np.float64))
    return D.astype(np.float32)


def _chunk_windows():
    """Per 128-row chunk of D, the even-aligned column support window."""
    Dp = _make_D(np.ones(TAPS, dtype=np.float32))
    wins = []
    for c in range(4):
        nz = np.nonzero(np.any(Dp[c * 128:(c + 1) * 128] != 0.0, axis=0))[0]
        j0 = int(nz.min()) & ~1
        j1 = min(HO, (int(nz.max()) + 2) & ~1)
        wins.append((j0, j1))
    cover = 0
    for a, b in zip(wins, wins[1:]):
        assert b[0] < a[1], f"windows must overlap for ordering: {wins}"
    assert wins[0][0] == 0 and wins[-1][1] == HO
    return wins


def _emit(tc, y, x, d, n_planes):
    nc = tc.nc
    f32 = mybir.dt.float32
    WIN = _chunk_windows()

    from contextlib import ExitStack
    with ExitStack() as ctx:
        xpool = ctx.enter_context(tc.tile_pool(name="xin", bufs=3))
        dpool = ctx.enter_context(tc.tile_pool(name="dconst", bufs=1))
        utpool = ctx.enter_context(tc.tile_pool(name="ut", bufs=4))
        ypool = ctx.enter_context(tc.tile_pool(name="yout", bufs=3))
        pspool = ctx.enter_context(tc.tile_pool(name="ps", bufs=1, space="PSUM"))

        # D in natural 128-row chunks: partition p = row 128*kc + p.  Serves
        # as the rhs of both phases (phase 1 contracts rows i, phase 2 rows w).
        d_sb = dpool.tile([128, 4, HO], d.dtype)
        nc.scalar.dma_start(d_sb[:], d.rearrange("(kc p) j -> p kc j", p=128))

        ut_dt = _MYBIR_DT[UT_MODE]

        # PSUM->SBUF casts carry a ~125-290ns fixed access latency per
        # instruction, so batch each plane's phase-1 result into ONE 2-bank
        # PSUM tile + ONE ACT cast, and each plane's phase-2 result into ONE
        # 1-bank tile + ONE DVE cast.  start=True arms the whole 2KiB bank
        # (lazy zero-on-first-matmul-write), so only the first sub-matmul of
        # each bank carries start=True and only the last carries stop=True.
        y_sbs = {}

        xns = {}

        def phase1(p):
            # x is pre-permuted on the host to [plane, q, kc, w] so each
            # partition line is one 2KiB contiguous DRAM run (128
            # descriptors per plane instead of 512); planes are loaded in
            # pairs to halve the fixed ~740ns dma_start issue cost.
            pp, pl2 = divmod(p, XPAIR)
            if pl2 == 0:
                xns[pp] = xpool.tile([128, XPAIR, 4, W], x.dtype, tag="x",
                                     bufs=8, name=f"xn{pp}")
                xv = x[pp * XPAIR:(pp + 1) * XPAIR].rearrange(
                    "pl q kc w -> q pl kc w")
                if pp == 0:
                    # head: smaller loads so the first matmul starts sooner
                    for h in range(XPAIR):
                        for kc in range(4):
                            nc.sync.dma_start(xns[pp][:, h, kc], xv[:, h, kc])
                else:
                    nc.sync.dma_start(xns[pp][:], xv)
            xn = xns[pp]

            ut_ps = pspool.tile([128, 4, HO], f32, tag="utps", bufs=3)
            for wc in range(4):
                for kc in range(4):
                    j0, j1 = WIN[kc]
                    nc.tensor.matmul(
                        ut_ps[:, wc, j0:j1],
                        xn[:, pl2, kc, wc * 128:(wc + 1) * 128],
                        d_sb[:, kc, j0:j1],
                        # banks are (wc 0,1) and (wc 2,3)
                        start=(kc == 0 and wc in (0, 2)),
                        stop=(kc == 3 and wc in (1, 3)),
                    )
            ut = utpool.tile([128, 4, HO], ut_dt, tag="ut")
            # split the PSUM->SBUF cast ~3:1 between ACT and DVE
            nc.scalar.copy(ut[:, 0:3, :], ut_ps[:, 0:3, :])
            nc.vector.tensor_copy(ut[:, 3, :], ut_ps[:, 3, :])
            return ut

        def phase2(p, ut):
            g, pl = divmod(p, GROUP)
            if pl == 0:
                y_sbs[g] = ypool.tile([128, GROUP, 2, WO], y.dtype, tag="y",
                                      name=f"y_sb{g}")
            y_sb = y_sbs[g]
            utv = ut[:].rearrange("q wc (j2 rr) -> q wc j2 rr", rr=2)
            y_ps = pspool.tile([128, 2, WO], f32, tag="yps", bufs=2)
            for rr in range(2):
                for wc in range(4):
                    j0, j1 = WIN[wc]
                    nc.tensor.matmul(
                        y_ps[:, rr, j0:j1],
                        utv[:, wc, :, rr],
                        d_sb[:, wc, j0:j1],
                        start=(wc == 0 and rr == 0),
                        stop=(wc == 3 and rr == 1),
                    )
            nc.vector.tensor_copy(y_sb[:, pl, :, :], y_ps[:])
            if pl == GROUP - 1:
                # y DRAM carries the SBUF layout (un-permuted on the host):
                # one 2KiB contiguous run per partition per store.  Issued on
                # the scalar queue so output bursts never head-of-line block
                # the x input stream on sync's queue.
                nc.scalar.dma_start(y[g], y_sb[:])
                del y_sbs[g]

        # software pipeline, distance 2: phase2(p-2) is emitted between
        # phase1(p) and phase1(p+1), giving plane p's ut cast two full
        # phase-1 blocks (~2.6us) to complete before the PE needs it.
        DIST = 2
        pending = []
        for p in range(n_planes):
            ut = phase1(p)
            pending.append((p, ut))
            if len(pending) > DIST:
                pp_, ut_ = pending.pop(0)
                phase2(pp_, ut_)
        for pp_, ut_ in pending:
            phase2(pp_, ut_)


def build_nc(n_planes=N_PLANES):
    nc = bacc.Bacc("TRN2", target_bir_lowering=False, debug=False)
    x = nc.dram_tensor("x", [n_planes, 128, 4, W], _MYBIR_DT[X_MODE],
                       kind="ExternalInput").ap()
    d = nc.dram_tensor("d", [H, HO], _MYBIR_DT[D_MODE],
                       kind="ExternalInput").ap()
    y = nc.dram_tensor("y", [n_planes // GROUP, 128, GROUP, 2, WO],
                       _MYBIR_DT[Y_MODE], kind="ExternalOutput").ap()
    with tile.TileContext(nc) as tc:
        _emit(tc, y, x, d, n_planes)
    nc.compile()
    return nc


_NC_CACHE = {}


def _get_nc(n_planes=N_PLANES):
    key = (n_planes, X_MODE, D_MODE, UT_MODE, Y_MODE)
    if key not in _NC_CACHE:
        _NC_CACHE[key] = build_nc(n_planes)
    return _NC_CACHE[key]


def kernel(x, kernel, **run_kwargs):
    x = np.asarray(x, dtype=np.float32)
    k = np.asarray(kernel, dtype=np.float32)
    B, C = x.shape[0], x.shape[1]
    assert x.shape == (B, C, H, W) and B * C == N_CORES * N_PLANES

    nc = _get_nc()
    d_in = _make_D(k).astype(_np_dt(D_MODE))
    # pre-permute x into the kernel's DMA-native layout:
    # [plane, q, kc, w] with row (128*kc + q) -> 2KiB contiguous per (plane, q)
    xs = x.reshape(N_CORES * N_PLANES, 4, 128, W).transpose(0, 2, 1, 3)
    xs = np.ascontiguousarray(xs, dtype=np.float32).astype(_np_dt(X_MODE))
    in_maps = [
        {"x": xs[c * N_PLANES:(c + 1) * N_PLANES], "d": d_in}
        for c in range(N_CORES)
    ]
    res = run_bass_kernel_spmd(nc, in_maps, core_ids=list(range(N_CORES)), **run_kwargs)
    # y arrives as [n_planes//GROUP, q, pl, rr, c]; un-permute on the host:
    # plane = GROUP*g + pl, row = 2*q + rr
    y = np.stack([np.asarray(r["y"], dtype=np.float32) for r in res.results])
    y = y.transpose(0, 1, 3, 2, 4, 5).reshape(N_CORES * N_PLANES, 128, 2, WO)
    out = np.ascontiguousarray(y).reshape(B, C, HO, WO)
    if run_kwargs:
        return out, res
    return out


# revision 18
# speedup vs baseline: 1.2073x; 1.0093x over previous
"""Trainium2 Bass kernel for AliasFreeSampling.

Reference op per (b, c) plane X (512x512):
  reflect-pad 32 -> 65-tap separable lowpass -> 2x2 average pool -> Y (256x256)

The whole per-plane operator is linear and separable, so it folds into a
single 512x256 matrix D (pad + conv + pool combined):  Y = D^T @ X @ D.

On the PE array (out = lhsT.T @ rhs, contraction over partitions):
  phase 1: U^T = X^T @ D    via lhsT = X-chunk   [K=i,128][M=w,128],
                                 rhs = D-chunk   [K=i,128][N=j,win]
           -> U^T [w, j] comes out directly, no transposes anywhere.
  phase 2: Y   = U @ D      via lhsT = U^T-chunk [K=w,128][M=j,128],
                                 rhs = D-chunk   [K=w,128][N=c,win]

D is banded (65-tap filter + 2x pool keeps it local): rows [128k, 128k+128)
only touch ~80-96 of the 256 output columns.  Both phases therefore stream
only each chunk's column window instead of all 256 columns (2112 vs 6144
PE cycles per plane).  PSUM accumulation with differing per-chunk windows
works because start=True arms the whole 2KiB PSUM bank for lazy
zero-on-first-matmul-write; the union of the windows covers every column,
so every byte the casts read was written.

Performance structure (per core: 8.4 MB fp8 in + 4.2 MB f16 out):
 - x is fp8 e3m4 (halves input DMA; PE takes fp8 stationary x f16 moving);
   D/U^T stay f16.  Measured end-to-end rel err 1.4e-2 vs the 2e-2 gate.
 - host pre-permutes x to [plane, q, kc, w] and receives y in the SBUF
   layout, so every DMA line is a 1-2 KiB contiguous DRAM run.
 - per-plane PSUM results are cast to SBUF in ONE 2-bank ACT copy (+ a
   1-bank DVE copy) instead of six small ones - the ~125-290 cycle
   PSUM-access latency is per instruction.
 - software pipeline distance 2: phase2(p-2) is emitted between phase1(p)
   and phase1(p+1), so the PE never waits on the PSUM->SBUF casts.
 - input loads (sync queue) and output stores (scalar queue) are issued on
   separate queues; DMA engines run at ~85% utilization and are the
   binding resource, with the PE floor (~940ns/plane) just below.

Sharding: pure data parallel - 256 (b,c) planes split as 32 planes on each
of the 8 NeuronCores; D is replicated; no cross-core communication.
"""

import numpy as np

import concourse.bacc as bacc
import concourse.bass as bass
import concourse.mybir as mybir
import concourse.tile as tile
from concourse.bass_utils import run_bass_kernel_spmd

N_CORES = 8
N_PLANES = 32        # planes per core
GROUP = 4            # planes per output-DMA batch
XPAIR = 2            # planes per input-DMA batch
H = W = 512
HO = WO = 256
PAD = 32
TAPS = 65

# dtype modes: x input, D filter matrix, U^T intermediate, y output
X_MODE = "f8e3"      # "f16" | "f8e3" (fp8 e3m4: halves input DMA bytes;
                     # PE allows mixed fp8 stationary x f16 moving)
D_MODE = "f16"
UT_MODE = "f16"
Y_MODE = "f16"

_MYBIR_DT = {
    "f16": mybir.dt.float16,
    "bf16": mybir.dt.bfloat16,
    "f8e3": mybir.dt.float8e3,
    "f32": mybir.dt.float32,
}


def _np_dt(mode):
    if mode == "f16":
        return np.float16
    if mode == "f32":
        return np.float32
    import ml_dtypes
    return {"bf16": ml_dtypes.bfloat16, "f8e3": ml_dtypes.float8_e3m4}[mode]


def _make_D(k: np.ndarray) -> np.ndarray:
    """Fold reflect-pad(32) + 65-tap conv + 2x avg-pool into one 512x256 map."""
    assert k.shape == (TAPS,)
    D = np.zeros((H, HO), dtype=np.float64)
    t = np.arange(TAPS)
    for j in range(HO):
        for r in (2 * j, 2 * j + 1):
            q = r + t - PAD
            i = np.where(q < 0, -q, np.where(q >= H, 2 * H - 2 - q, q))
            np.add.at(D[:, j], i, 0.5 * k.astype(np.float64))
    return D.astype(np.float32)


def _chunk_windows():
    """Per 128-row chunk of D, the even-aligned column support window."""
    Dp = _make_D(np.ones(TAPS, dtype=np.float32))
    wins = []
    for c in range(4):
        nz = np.nonzero(np.any(Dp[c * 128:(c + 1) * 128] != 0.0, axis=0))[0]
        j0 = int(nz.min()) & ~1
        j1 = min(HO, (int(nz.max()) + 2) & ~1)
        wins.append((j0, j1))
    cover = 0
    for a, b in zip(wins, wins[1:]):
        assert b[0] < a[1], f"windows must overlap for ordering: {wins}"
    assert wins[0][0] == 0 and wins[-1][1] == HO
    return wins


def _emit(tc, y, x, d, n_planes):
    nc = tc.nc
    f32 = mybir.dt.float32
    WIN = _chunk_windows()

    from contextlib import ExitStack
    with ExitStack() as ctx:
        xpool = ctx.enter_context(tc.tile_pool(name="xin", bufs=3))
        dpool = ctx.enter_context(tc.tile_pool(name="dconst", bufs=1))
        utpool = ctx.enter_context(tc.tile_pool(name="ut", bufs=6))
        ypool = ctx.enter_context(tc.tile_pool(name="yout", bufs=4))
        pspool = ctx.enter_context(tc.tile_pool(name="ps", bufs=1, space="PSUM"))

        # D in natural 128-row chunks: partition p = row 128*kc + p.  Serves
        # as the rhs of both phases (phase 1 contracts rows i, phase 2 rows w).
        d_sb = dpool.tile([128, 4, HO], d.dtype)
        nc.scalar.dma_start(d_sb[:], d.rearrange("(kc p) j -> p kc j", p=128))

        ut_dt = _MYBIR_DT[UT_MODE]

        # PSUM->SBUF casts carry a ~125-290ns fixed access latency per
        # instruction, so batch each plane's phase-1 result into ONE 2-bank
        # PSUM tile + ONE ACT cast, and each plane's phase-2 result into ONE
        # 1-bank tile + ONE DVE cast.  start=True arms the whole 2KiB bank
        # (lazy zero-on-first-matmul-write), so only the first sub-matmul of
        # each bank carries start=True and only the last carries stop=True.
        y_sbs = {}

        xns = {}

        def phase1(p):
            # x is pre-permuted on the host to [plane, q, kc, w] so each
            # partition line is one 2KiB contiguous DRAM run (128
            # descriptors per plane instead of 512); planes are loaded in
            # pairs to halve the fixed ~740ns dma_start issue cost.
            pp, pl2 = divmod(p, XPAIR)
            if pl2 == 0:
                xns[pp] = xpool.tile([128, XPAIR, 4, W], x.dtype, tag="x",
                                     bufs=10, name=f"xn{pp}")
                xv = x[pp * XPAIR:(pp + 1) * XPAIR].rearrange(
                    "pl q kc w -> q pl kc w")
                if pp == 0:
                    # head: smaller loads so the first matmul starts sooner
                    for h in range(XPAIR):
                        for kc in range(4):
                            nc.sync.dma_start(xns[pp][:, h, kc], xv[:, h, kc])
                else:
                    nc.sync.dma_start(xns[pp][:], xv)
            xn = xns[pp]

            ut_ps = pspool.tile([128, 4, HO], f32, tag="utps", bufs=3)
            for wc in range(4):
                for kc in range(4):
                    j0, j1 = WIN[kc]
                    nc.tensor.matmul(
                        ut_ps[:, wc, j0:j1],
                        xn[:, pl2, kc, wc * 128:(wc + 1) * 128],
                        d_sb[:, kc, j0:j1],
                        # banks are (wc 0,1) and (wc 2,3)
                        start=(kc == 0 and wc in (0, 2)),
                        stop=(kc == 3 and wc in (1, 3)),
                    )
            ut = utpool.tile([128, 4, HO], ut_dt, tag="ut")
            # split the PSUM->SBUF cast ~3:1 between ACT and DVE
            nc.scalar.copy(ut[:, 0:3, :], ut_ps[:, 0:3, :])
            nc.vector.tensor_copy(ut[:, 3, :], ut_ps[:, 3, :])
            return ut

        def phase2(p, ut):
            g, pl = divmod(p, GROUP)
            if pl == 0:
                y_sbs[g] = ypool.tile([128, GROUP, 2, WO], y.dtype, tag="y",
                                      name=f"y_sb{g}")
            y_sb = y_sbs[g]
            utv = ut[:].rearrange("q wc (j2 rr) -> q wc j2 rr", rr=2)
            y_ps = pspool.tile([128, 2, WO], f32, tag="yps", bufs=2)
            for rr in range(2):
                for wc in range(4):
                    j0, j1 = WIN[wc]
                    nc.tensor.matmul(
                        y_ps[:, rr, j0:j1],
                        utv[:, wc, :, rr],
                        d_sb[:, wc, j0:j1],
                        start=(wc == 0 and rr == 0),
                        stop=(wc == 3 and rr == 1),
                    )
            nc.vector.tensor_copy(y_sb[:, pl, :, :], y_ps[:])
            # y DRAM carries the SBUF layout (un-permuted on the host): 2KiB
            # contiguous per partition per store.  Issued on the scalar queue
            # so output bursts never head-of-line block the x input stream.
            # The last group stores per-plane so the final store (and the
            # exit drain behind it) only waits on one plane's cast.
            last_group = (g == n_planes // GROUP - 1)
            if last_group:
                nc.scalar.dma_start(y[g][:, pl], y_sb[:, pl])
                if pl == GROUP - 1:
                    del y_sbs[g]
            elif pl == GROUP - 1:
                nc.scalar.dma_start(y[g], y_sb[:])
                del y_sbs[g]

        # software pipeline, distance 2: phase2(p-2) is emitted between
        # phase1(p) and phase1(p+1), giving plane p's ut cast two full
        # phase-1 blocks (~2.6us) to complete before the PE needs it.
        DIST = 2
        pending = []
        for p in range(n_planes):
            ut = phase1(p)
            pending.append((p, ut))
            if len(pending) > DIST:
                pp_, ut_ = pending.pop(0)
                phase2(pp_, ut_)
        for pp_, ut_ in pending:
            phase2(pp_, ut_)


def build_nc(n_planes=N_PLANES):
    nc = bacc.Bacc("TRN2", target_bir_lowering=False, debug=False)
    x = nc.dram_tensor("x", [n_planes, 128, 4, W], _MYBIR_DT[X_MODE],
                       kind="ExternalInput").ap()
    d = nc.dram_tensor("d", [H, HO], _MYBIR_DT[D_MODE],
                       kind="ExternalInput").ap()
    y = nc.dram_tensor("y", [n_planes // GROUP, 128, GROUP, 2, WO],
                       _MYBIR_DT[Y_MODE], kind="ExternalOutput").ap()
    with tile.TileContext(nc) as tc:
        _emit(tc, y, x, d, n_planes)
    nc.compile()
    return nc


_NC_CACHE = {}


def _get_nc(n_planes=N_PLANES):
    key = (n_planes, X_MODE, D_MODE, UT_MODE, Y_MODE)
    if key not in _NC_CACHE:
        _NC_CACHE[key] = build_nc(n_planes)
    return _NC_CACHE[key]


def kernel(x, kernel, **run_kwargs):
    x = np.asarray(x, dtype=np.float32)
    k = np.asarray(kernel, dtype=np.float32)
    B, C = x.shape[0], x.shape[1]
    assert x.shape == (B, C, H, W) and B * C == N_CORES * N_PLANES

    nc = _get_nc()
    d_in = _make_D(k).astype(_np_dt(D_MODE))
    # pre-permute x into the kernel's DMA-native layout:
    # [plane, q, kc, w] with row (128*kc + q) -> 2KiB contiguous per (plane, q)
    xs = x.reshape(N_CORES * N_PLANES, 4, 128, W).transpose(0, 2, 1, 3)
    xs = np.ascontiguousarray(xs, dtype=np.float32).astype(_np_dt(X_MODE))
    in_maps = [
        {"x": xs[c * N_PLANES:(c + 1) * N_PLANES], "d": d_in}
        for c in range(N_CORES)
    ]
    res = run_bass_kernel_spmd(nc, in_maps, core_ids=list(range(N_CORES)), **run_kwargs)
    # y arrives as [n_planes//GROUP, q, pl, rr, c]; un-permute on the host:
    # plane = GROUP*g + pl, row = 2*q + rr
    y = np.stack([np.asarray(r["y"], dtype=np.float32) for r in res.results])
    y = y.transpose(0, 1, 3, 2, 4, 5).reshape(N_CORES * N_PLANES, 128, 2, WO)
    out = np.ascontiguousarray(y).reshape(B, C, HO, WO)
    if run_kwargs:
        return out, res
    return out
